# revision 10
# baseline (speedup 1.0000x reference)
"""TRN2 Bass kernel for nn_ConvLayer_75239237091621 (convolutional GP layer).

Math restructuring (host precompute is O(M^3), device does the O(P*N*M^2) work):
  Kuf[m,c] = variance * exp(-0.5*(z2[m] + x2[c] - 2*zs_m.xs_c))
           = dz[m] * Kt[m,c],   Kt = exp(Zs @ Xs^T - 0.5*x2)   (x2 folded into GEMM)
  mean_c   = (alphaz^T Kt)_c,             alphaz = dz * (Kuu^-1 q_mu)
  var_c    = variance + (Kt^T Cz Kt)_cc,  Cz = diag(dz) (Kuu^-1 qS Kuu^-1 - Kuu^-1) diag(dz)

Device (per core, cols = P*N/8 = 4608 flattened patch-points, col tiles of 384):
  d2-GEMM   Kt_psum = ZA.T @ XA      (fp32r, K=27: 25 dims + x2 hi/lo rows)
  exp       one batched ACT op -> fp32r Kt  (3-bank strided psum read)
  T-GEMM    T = Cz @ Kt              (fp32r, 3-bank strided psum)
  mean-GEMM alphaz^T @ Kt -> psum row 0   (fp32r)
  E = Kt.*T (one batched DVE op), colsum ones^T @ E -> psum row 32
  finish    one tensor_scalar_add with per-partition [0, vv] -> staging rows
Sharding: patch-point columns (P-major) split 8 ways; gather = concat on host.
"""
import sys

sys.path.insert(0, "/opt/trn_rl_repo")

import numpy as np
import ml_dtypes

import concourse.bass as bass
import concourse.tile as tile
from concourse import bacc, mybir
from concourse.bass_utils import run_bass_kernel_spmd

dt = mybir.dt

# geometry (hardcoded per problem spec)
N = 64
H = W = 28
FH = FW = 5
OH = OW = 24
P = OH * OW            # 576
L = FH * FW            # 25
M = 384                # inducing points
JITTER = 1e-6
NCORES = 8
COLS = P * N // NCORES  # 4608 patch-point columns per core
CT = 384               # column tile (fp32r needs >=256 for 1 cyc/row)
NCT = COLS // CT       # 12
KB = M // 128          # 3 k/m blocks
KA = L + 2             # 27 GEMM contraction rows (25 dims + x2_hi + x2_lo)
XBLK = 3               # XA packed into 3 row-blocks of 32 partitions (base 0/32/64 only)
BCOLS = COLS // XBLK   # 1536 columns per packed block (= 4 col tiles)

_CACHE = {}


def _build(reps=1):
    nc = bacc.Bacc("TRN2", target_bir_lowering=False, debug=False,
                   enable_asserts=True, num_devices=NCORES)

    za_d = nc.dram_tensor("za", (32 * XBLK, M), dt.float32r, kind="ExternalInput").ap()
    xa_d = nc.dram_tensor("xa", (32 * XBLK, BCOLS), dt.float32r,
                          kind="ExternalInput").ap()
    cz_d = nc.dram_tensor("cz", (M, M), dt.float32r, kind="ExternalInput").ap()
    az_d = nc.dram_tensor("az", (M, 1), dt.float32r, kind="ExternalInput").ap()
    vv_d = nc.dram_tensor("vv", (1, 1), dt.float32, kind="ExternalInput").ap()
    ones_d = nc.dram_tensor("ones", (128, 1), dt.float32r, kind="ExternalInput").ap()
    mean_d = nc.dram_tensor("mean", (1, COLS), dt.float32, kind="ExternalOutput").ap()
    var_d = nc.dram_tensor("var", (1, COLS), dt.float32, kind="ExternalOutput").ap()

    with tile.TileContext(nc) as tc:
        with tc.tile_pool(name="consts", bufs=1) as consts, \
             tc.tile_pool(name="kt", bufs=2) as kt_pool, \
             tc.tile_pool(name="ebuf", bufs=2) as e_pool, \
             tc.tile_pool(name="ps_d2", bufs=1, space="PSUM") as ps_d2, \
             tc.tile_pool(name="ps_t", bufs=1, space="PSUM") as ps_t, \
             tc.tile_pool(name="ps_m", bufs=1, space="PSUM") as ps_m, \
             tc.tile_pool(name="ps_v", bufs=1, space="PSUM") as ps_v:

            za_sb = consts.tile([32 * XBLK, M], dt.float32r)
            nc.sync.dma_start(za_sb[:], za_d)
            xa_sb = consts.tile([32 * XBLK, BCOLS], dt.float32r)
            nc.sync.dma_start(xa_sb[:], xa_d)
            cz_sb = consts.tile([128, KB, M], dt.float32r)
            nc.sync.dma_start(cz_sb[:], cz_d.rearrange("(a p) m -> p a m", p=128))
            az_sb = consts.tile([128, KB], dt.float32r)
            nc.sync.dma_start(az_sb[:], az_d.rearrange("(a p) one -> p (a one)", p=128))
            vv_sb = consts.tile([1, 1], dt.float32)
            nc.sync.dma_start(vv_sb[:], vv_d)
            ones_sb = consts.tile([128, 1], dt.float32r)
            nc.sync.dma_start(ones_sb[:], ones_d)
            # persistent output staging: partition 0 = mean, partition 32 = var
            out_sb = consts.tile([33, COLS], dt.float32)

            for _ in range(reps):
                for ct in range(NCT):
                    blk = ct // (BCOLS // CT)           # packed row-block
                    boff = (ct % (BCOLS // CT)) * CT    # column offset in block
                    xa_ap = xa_sb[32 * blk:32 * blk + KA, boff:boff + CT]

                    # d2-GEMM into a 3-bank strided psum, one batched exp
                    pd3 = ps_d2.tile([128, KB, 512], dt.float32, tag="pd3")
                    for kb in range(KB):
                        nc.tensor.matmul(
                            pd3[:, kb, 0:CT],
                            za_sb[32 * blk:32 * blk + KA, bass.ts(kb, 128)],
                            xa_ap, start=True, stop=True)
                    kt_r = kt_pool.tile([128, KB, CT], dt.float32r, tag="kt_r")
                    nc.scalar.activation(kt_r[:], pd3[:, :, 0:CT],
                                         func=mybir.ActivationFunctionType.Exp)

                    # mean GEMM (fp32r, accumulate over kb)
                    pm = ps_m.tile([1, 512], dt.float32, tag="pm")
                    for kb in range(KB):
                        nc.tensor.matmul(pm[0:1, 0:CT], az_sb[:, kb:kb + 1],
                                         kt_r[:, kb, :],
                                         start=(kb == 0), stop=(kb == KB - 1))

                    # T-GEMM into 3-bank strided psum, one batched E-mul
                    pt3 = ps_t.tile([128, KB, 512], dt.float32, tag="pt3")
                    for mb in range(KB):
                        for kb in range(KB):
                            nc.tensor.matmul(pt3[:, mb, 0:CT],
                                             cz_sb[:, kb, bass.ts(mb, 128)],
                                             kt_r[:, kb, :],
                                             start=(kb == 0), stop=(kb == KB - 1))
                    e_r = e_pool.tile([128, KB, CT], dt.float32r, tag="e")
                    nc.vector.tensor_mul(e_r[:], kt_r[:].bitcast(dt.float32),
                                         pt3[:, :, 0:CT])
                    pv = ps_v.tile([1, 512], dt.float32, tag="pv")
                    for mb in range(KB):
                        nc.tensor.matmul(pv[0:1, 0:CT], ones_sb[:],
                                         e_r[:, mb, :],
                                         start=(mb == 0), stop=(mb == KB - 1))

                    # finish: mean copy on ACT, var = colsum + vv on DVE
                    nc.scalar.copy(out_sb[0:1, bass.ts(ct, CT)], pm[0:1, 0:CT])
                    nc.vector.tensor_scalar_add(
                        out_sb[32:33, bass.ts(ct, CT)],
                        pv[0:1, 0:CT], vv_sb[:])

                nc.sync.dma_start(mean_d[:], out_sb[0:1, :])
                nc.sync.dma_start(var_d[:], out_sb[32:33, :])

    nc.compile()
    return nc


def _precompute(ND_X, Z, q_mu, q_sqrt, variance, lengthscale):
    """Host-side O(M^3) prep + patch extraction; float64 for stability."""
    variance = float(np.asarray(variance))
    lengthscale = float(np.asarray(lengthscale))

    Zs = np.asarray(Z, np.float64) / lengthscale
    z2 = (Zs * Zs).sum(1)
    d2zz = np.maximum(z2[:, None] + z2[None, :] - 2.0 * (Zs @ Zs.T), 0.0)
    Kuu = variance * np.exp(-0.5 * d2zz) + JITTER * np.eye(M)
    Kuu_inv = np.linalg.inv(Kuu)
    alpha = Kuu_inv @ np.asarray(q_mu, np.float64)
    Ls = np.tril(np.asarray(q_sqrt, np.float64)[0])
    C = Kuu_inv @ (Ls @ Ls.T) @ Kuu_inv - Kuu_inv

    dz = variance * np.exp(-0.5 * z2)
    alphaz = (dz * alpha[:, 0]).reshape(M, 1)
    Cz = dz[:, None] * C * dz[None, :]

    # patch extraction: (P, N, L) row-major (fh, fw) like the reference
    x = np.asarray(ND_X, np.float64).reshape(N, H, W)
    i_idx = np.arange(OH)[:, None] + np.arange(FH)[None, :]
    j_idx = np.arange(OW)[:, None] + np.arange(FW)[None, :]
    w = x[:, i_idx][:, :, :, j_idx]              # (N, OH, FH, OW, FW)
    w = np.transpose(w, (1, 3, 0, 2, 4))         # (OH, OW, N, FH, FW)
    X_all = w.reshape(P * N, L) / lengthscale    # col index c = p*N + n
    x2 = (X_all * X_all).sum(1)

    # GEMM rows 25/26 carry -0.5*x2 split hi/lo so fp32r rounding stays exact
    mhalf_x2 = -0.5 * x2
    x2_hi = mhalf_x2.astype(ml_dtypes.bfloat16).astype(np.float64)
    x2_lo = mhalf_x2 - x2_hi

    za = np.zeros((32 * XBLK, M), np.float32)
    for b in range(XBLK):
        za[32 * b:32 * b + L] = Zs.T
        za[32 * b + L:32 * b + KA] = 1.0
    xs_all = np.empty((KA, P * N), np.float32)
    xs_all[:L] = X_all.T
    xs_all[L] = x2_hi
    xs_all[L + 1] = x2_lo

    return dict(
        za=za,
        xs_all=xs_all,
        cz=Cz.astype(np.float32),
        az=alphaz.astype(np.float32),
        vv=np.full((1, 1), variance, np.float32),
        ones=np.ones((128, 1), np.float32),
    )


def _pack_xa(xs_core):
    """(27, COLS) -> (96, BCOLS): 3 col-blocks stacked at 32-partition offsets."""
    out = np.zeros((32 * XBLK, BCOLS), np.float32)
    for b in range(XBLK):
        out[32 * b:32 * b + KA] = xs_core[:, b * BCOLS:(b + 1) * BCOLS]
    return out


def kernel(ND_X, Z, q_mu, q_sqrt, variance, lengthscale):
    pre = _precompute(ND_X, Z, q_mu, q_sqrt, variance, lengthscale)

    if "nc" not in _CACHE:
        _CACHE["nc"] = _build()
    nc = _CACHE["nc"]

    in_maps = []
    for c in range(NCORES):
        cs = slice(c * COLS, (c + 1) * COLS)
        in_maps.append({
            "za": pre["za"], "cz": pre["cz"], "az": pre["az"], "vv": pre["vv"],
            "ones": pre["ones"],
            "xa": _pack_xa(pre["xs_all"][:, cs]),
        })

    res = run_bass_kernel_spmd(nc, in_maps, core_ids=list(range(NCORES)))

    mean_c = np.concatenate([r["mean"][0] for r in res.results])  # (P*N,)
    var_c = np.concatenate([r["var"][0] for r in res.results])
    NP_mean = mean_c.reshape(P, N).T.astype(np.float32, copy=False)
    NP_var = var_c.reshape(P, N).T.astype(np.float32, copy=False)
    return np.ascontiguousarray(NP_mean), np.ascontiguousarray(NP_var)
